# revision 6
# baseline (speedup 1.0000x reference)
"""Causal single-head attention on 8 Trainium2 NeuronCores.

Problem: x[8, 2048, 1024] -> out[8, 2048, 64]
  q/k/v = x @ W{q,k,v} + b{q,k,v};  out = softmax(causal(q k^T / 8)) v

Sharding: data-parallel over batch; core b computes batch element b.

Per-core layout strategy (T=2048, D=1024, H=64):
  - host sends x[b]^T as xt [D, T] so the D contraction sits on partitions
  - QKV: lhsT=[Wq|Wk] chunk [128d, 128] (full array), rhs=xt [128d, 512t]
    -> psum [128(qh|kh), 512t]; biases added during PSUM->SBUF copy,
    producing qT/kT [64h, T].  V same with lhsT=Wv (M=64), then PE-transposed
    to natural v [128t, 64h] tiles with a ones column appended -> [128, 65].
  - S^T strips per j-chunk: psum[128j, i] = kT_chunk.T @ qT (K=64)
  - exp: ACT reads psum strip, writes P strip (f32r) with scale=1/8 fused
  - causal: only the diagonal 128-block of each strip needs masking
    (affine_select); columns i < 128*jt are never computed.
  - PV: psum_out[65, i] += [v_jt | 1].T @ P_strip ; row 64 accumulates the
    softmax denominator for free.  i processed in two 1024-halves.
  - out: PE-transpose [65,128] blocks -> [128, 65]; divide by denominator
    (per-partition scalar after transpose); single DMA to out [T, H].

All matmuls run in float32r (1 col/cycle on TRN2 vs 4 for float32).
"""

import os
from contextlib import ExitStack

import numpy as np

import concourse.bass as bass
import concourse.bacc as bacc
import concourse.mybir as mybir
import concourse.tile as tile
from concourse.bass_utils import run_bass_kernel_spmd

F32 = mybir.dt.float32
F32R = mybir.dt.float32r
AF = mybir.ActivationFunctionType
ALU = mybir.AluOpType

T = 2048
D = 1024
H = 64
NB = 8
DC = D // 128     # 8 contraction chunks
NJT = T // 128    # 16 j-chunks (also 16 t-tiles)
IW = 1024         # i-half width
SCALE = 1.0 / 8.0  # 1/sqrt(H)

_CACHE: dict = {}


def _emit_qkv_half(nc, th, xt_sb, wqk_sb, wv_sb, bqk_sb, bv_sb,
                   qT, kT, vT, v_sb, ident, work_ps):
    """QKV for t in [th*1024, (th+1)*1024)."""
    lo = th * IW
    ps_qk = work_ps.tile([128, IW], F32, tag="work")
    for t2 in range(2):
        psl = slice(t2 * 512, t2 * 512 + 512)
        sl = slice(lo + t2 * 512, lo + t2 * 512 + 512)
        for c in range(DC):
            nc.tensor.matmul(
                ps_qk[:, psl], wqk_sb[:, c, :], xt_sb[:, c, sl],
                start=(c == 0), stop=(c == DC - 1),
            )
    nc.vector.tensor_scalar(
        out=qT[:, lo:lo + IW], in0=ps_qk[0:64, :],
        scalar1=bqk_sb[0:64, :], scalar2=None, op0=ALU.add,
    )
    nc.vector.tensor_scalar(
        out=kT[:, lo:lo + IW], in0=ps_qk[64:128, :],
        scalar1=bqk_sb[64:128, :], scalar2=None, op0=ALU.add,
    )

    ps_v = work_ps.tile([64, IW], F32, tag="work")
    for t2 in range(2):
        psl = slice(t2 * 512, t2 * 512 + 512)
        sl = slice(lo + t2 * 512, lo + t2 * 512 + 512)
        for c in range(DC):
            nc.tensor.matmul(
                ps_v[:, psl], wv_sb[:, c, :], xt_sb[:, c, sl],
                start=(c == 0), stop=(c == DC - 1),
            )
    nc.vector.tensor_scalar(
        out=vT[:, lo:lo + IW], in0=ps_v[:],
        scalar1=bv_sb[:], scalar2=None, op0=ALU.add,
    )

    # transpose v^T [64, 128]-tiles -> natural v [128, 64] tiles
    ps_t = work_ps.tile([128, 8, H], F32, tag="work")
    for j2 in range(8):
        jt = th * 8 + j2
        nc.tensor.transpose(
            ps_t[:, j2, :].bitcast(F32R),
            vT[:, jt * 128:(jt + 1) * 128],
            ident[0:64, 0:64],
        )
    nc.vector.tensor_copy(v_sb[:, th * 8:(th + 1) * 8, 0:H], ps_t[:, :, :])


def _emit_attn_half(nc, ih, qT, kT, v_sb, ident, work_ps, out_ps,
                    ppool, otpool, out_nat):
    """S/exp/PV for i in [ih*1024, (ih+1)*1024), all causal j."""
    jt_max = 8 * (ih + 1)
    ps_o = out_ps.tile([66, IW], F32, tag="out")
    for jt in range(jt_max):
        off_abs = max(128 * jt, ih * IW)
        off = off_abs - ih * IW
        ps_s = work_ps.tile([128, IW], F32, tag="work")
        for s in range(2):
            a, b = max(off, s * 512), (s + 1) * 512
            if a < b:
                nc.tensor.matmul(
                    ps_s[:, a:b],
                    kT[:, jt * 128:(jt + 1) * 128],
                    qT[:, ih * IW + a: ih * IW + b],
                    start=True, stop=True,
                )
        P = ppool.tile([128, IW], F32R, tag="P")
        nc.scalar.activation(
            out=P[:, off:IW], in_=ps_s[:, off:IW], func=AF.Exp, scale=SCALE,
        )
        if off_abs == 128 * jt:
            # diagonal block: keep i >= j  (i = off_abs + f, j = 128*jt + p)
            nc.gpsimd.affine_select(
                out=P[:, off:off + 128], in_=P[:, off:off + 128],
                compare_op=ALU.is_ge, fill=0.0,
                base=0, pattern=[[1, 128]], channel_multiplier=-1,
            )
        for s in range(2):
            a, b = max(off, s * 512), (s + 1) * 512
            if a < b:
                jt_last = min(jt_max - 1, (ih * IW + b) // 128 - 1)
                nc.tensor.matmul(
                    ps_o[:, a:b], v_sb[:, jt, :], P[:, a:b],
                    start=(jt == 0), stop=(jt == jt_last),
                )

    # drain: transpose [65, 128] blocks -> [128, 65] and stash
    oT = otpool.tile([66, IW], F32R, tag="oT")
    nc.vector.tensor_copy(oT[:], ps_o[:])
    ps_n = work_ps.tile([128, 8, 128], F32, tag="work")
    for t2 in range(8):
        nc.tensor.transpose(
            ps_n[:, t2, 0:66].bitcast(F32R),
            oT[:, t2 * 128:(t2 + 1) * 128],
            ident[:, :],
        )
    nc.vector.tensor_copy(
        out_nat[:, ih * 8:(ih + 1) * 8, :], ps_n[:, :, 0:66],
    )


def _build():
    nc = bacc.Bacc("TRN2", target_bir_lowering=False, debug=False, num_devices=8)
    xt = nc.dram_tensor("xt", [D, T], F32R, kind="ExternalInput")
    wqk = nc.dram_tensor("wqk", [D, 128], F32R, kind="ExternalInput")
    wv = nc.dram_tensor("wv", [D, H], F32R, kind="ExternalInput")
    bqk = nc.dram_tensor("bqk", [128, 1], F32, kind="ExternalInput")
    bv = nc.dram_tensor("bv", [H, 1], F32, kind="ExternalInput")
    ident66 = nc.dram_tensor("ident66", [66, 66], F32R, kind="ExternalInput")
    vtail = nc.dram_tensor("vtail", [128, NJT, 2], F32R, kind="ExternalInput")
    out = nc.dram_tensor("out", [T, H], F32, kind="ExternalOutput")

    with ExitStack() as ctx:
        tc = ctx.enter_context(tile.TileContext(nc))
        const = ctx.enter_context(tc.tile_pool(name="const", bufs=1))
        big = ctx.enter_context(tc.tile_pool(name="big", bufs=1))
        ppool = ctx.enter_context(tc.tile_pool(name="ppool", bufs=3))
        otpool = ctx.enter_context(tc.tile_pool(name="otpool", bufs=2))
        work_ps = ctx.enter_context(
            tc.tile_pool(name="work_ps", bufs=3, space="PSUM"))
        out_ps = ctx.enter_context(
            tc.tile_pool(name="out_ps", bufs=1, space="PSUM"))

        # constants / weights
        wqk_sb = const.tile([128, DC, 128], F32R)
        nc.sync.dma_start(
            out=wqk_sb[:], in_=wqk.rearrange("(c p) m -> p c m", p=128))
        wv_sb = const.tile([128, DC, H], F32R)
        nc.sync.dma_start(
            out=wv_sb[:], in_=wv.rearrange("(c p) m -> p c m", p=128))
        bqk_sb = const.tile([128, 1], F32)
        nc.sync.dma_start(out=bqk_sb[:], in_=bqk[:])
        bv_sb = const.tile([H, 1], F32)
        nc.sync.dma_start(out=bv_sb[:], in_=bv[:])
        ident = const.tile([66, 66], F32R)
        nc.sync.dma_start(out=ident[:], in_=ident66[:])

        # x^T resident in SBUF, f32r (rounded during SWDGE cast-DMA)
        xt_sb = big.tile([128, DC, T], F32R)
        for th in range(2):
            for c in range(DC):
                nc.sync.dma_start(
                    out=xt_sb[:, c, th * IW:(th + 1) * IW],
                    in_=xt[c * 128:(c + 1) * 128, th * IW:(th + 1) * IW],
                )

        qT = big.tile([64, T], F32R)
        kT = big.tile([64, T], F32R)
        vT = big.tile([64, T], F32R)
        v_sb = big.tile([128, NJT, H + 2], F32R)
        nc.sync.dma_start(out=v_sb[:, :, H:H + 2], in_=vtail[:])
        out_nat = big.tile([128, NJT, H + 2], F32)

        args = (qT, kT, vT, v_sb, ident, work_ps)
        _emit_qkv_half(nc, 0, xt_sb, wqk_sb, wv_sb, bqk_sb, bv_sb, *args)
        _emit_attn_half(nc, 0, qT, kT, v_sb, ident, work_ps, out_ps,
                        ppool, otpool, out_nat)
        _emit_qkv_half(nc, 1, xt_sb, wqk_sb, wv_sb, bqk_sb, bv_sb, *args)
        _emit_attn_half(nc, 1, qT, kT, v_sb, ident, work_ps, out_ps,
                        ppool, otpool, out_nat)

        # normalize: out[:, tt, 0:64] /= out[:, tt, 64]
        recip = const.tile([128, NJT], F32)
        nc.vector.reciprocal(recip[:], out_nat[:, :, H])
        for tt in range(NJT):
            nc.vector.tensor_scalar_mul(
                out_nat[:, tt, 0:H], out_nat[:, tt, 0:H], recip[:, tt:tt + 1])
        nc.sync.dma_start(
            out=out.rearrange("(tt p) h -> p tt h", p=128),
            in_=out_nat[:, :, 0:H],
        )

    nc.compile()
    return nc


def _get_nc():
    if "nc" not in _CACHE:
        _CACHE["nc"] = _build()
    return _CACHE["nc"]


def kernel(x, Wq, bq, Wk, bk, Wv, bv):
    x = np.ascontiguousarray(np.asarray(x, dtype=np.float32))
    Wq = np.asarray(Wq, dtype=np.float32)
    Wk = np.asarray(Wk, dtype=np.float32)
    Wv = np.ascontiguousarray(np.asarray(Wv, dtype=np.float32))
    bq = np.asarray(bq, dtype=np.float32)
    bk = np.asarray(bk, dtype=np.float32)
    bv = np.asarray(bv, dtype=np.float32)

    wqk = np.ascontiguousarray(np.concatenate([Wq, Wk], axis=1))
    bqk = np.ascontiguousarray(np.concatenate([bq, bk])[:, None])
    bv_ = np.ascontiguousarray(bv[:, None])

    ident66 = np.eye(66, dtype=np.float32)
    vtail = np.zeros((128, NJT, 2), dtype=np.float32)
    vtail[:, :, 0] = 1.0
    in_maps = []
    for b in range(NB):
        in_maps.append({
            "xt": np.ascontiguousarray(x[b].T),
            "wqk": wqk,
            "wv": Wv,
            "bqk": bqk,
            "bv": bv_,
            "ident66": ident66,
            "vtail": vtail,
        })

    nc = _get_nc()
    trace = bool(int(os.environ.get("KTRACE", "0")))
    res = run_bass_kernel_spmd(
        nc, in_maps, core_ids=list(range(NB)), trace=trace,
    )
    if trace:
        _CACHE["exec_time_ns"] = res.exec_time_ns
        _CACHE["results"] = res
    return np.stack([r["out"] for r in res.results])
